# revision 9
# baseline (speedup 1.0000x reference)
"""AdaptiveCornerLoss on 8 TRN2 NeuronCores — batch-parallel Bass/Tile kernel.

Shapes (hardcoded): B=64, N=16384, C=6, M=128 corners. 8 cores, 8 samples/core.

Math:
  focal    = u^2 * ce  with  y=(1-2t)*x, ce=softplus(y)=y+ln(1+e^-y),
             u=sigmoid(y)  =>  u^2 = exp(-2*ln(1+e^{-y}))
  d2(n,m)  = |p|^2 + |c|^2 - 2 p.c   (fp16 matmul; point rows [px,py,pz,|p|^2,1]
             vs corner rows [-2cx,-2cy,-2cz,1,|c|^2+pen])
  w        = exp(-10*sqrt(max(min_m d2, 1e-12))) via exp/ln on ACT

v2 design notes (measured on hw via probe):
  * PE output port is the hard floor: 128 fp32 PSUM values per 0.833ns
    cycle regardless of matmul shape (~82us/core for N*Mk d2 values).
    One 128-wide stationary [40,128] per pair of A/B groups; LDW (143ns)
    hides under the 320ns streams.
  * Corners split into A-half [0:Mh) / B-half [Mh:Mk); PE emits them as
    SEPARATE 384-col PSUM groups (8-slot ring of 512-col slots). This
    decouples ACT (copies A and/or B to SBUF bf16) from DVE (pairmin
    TT-min), since a TT may read only ONE operand from PSUM and GPSIMD
    cannot read PSUM at all (nor do min).
  * Drain windows = 2 pairs (4 groups, 16 chunks). Two flavors:
      P2 window: ACT copies all 4 groups -> bf16; DVE TT-min in 2x mode.
      P1 window: ACT copies A-groups only; DVE TT-min(B-PSUM, A-sbuf).
    The mix balances ACT vs DVE load (tunable P2MOD/P2CNT).
  * bf16 min-tree per half-sample batch on DVE (2x mode), final 3-wide
    tensor_reduce -> minsq fp32.
  * GPSIMD (Pool) handles focal ce/fo and the epilogue multiply; ACT the
    exp/ln chains.
Outputs per core: per-partition partial sums [128,8]: col0 = sum(focal),
cols 4:6 = half-sums of focal*w; host reduces and forms the 3 losses.
"""

import sys

sys.path.insert(0, "/opt/trn_rl_repo")

import numpy as np

import concourse.bass as bass
import concourse.mybir as mybir
from concourse import tile
from concourse.bass_utils import run_bass_kernel_spmd

# ---- inlined wait-splitter (this neuronx-cc build allows at most ONE sync
# wait per instruction; Tile emits multi-waits freely, so excess waits move
# onto nofuse NOPs on the same engine immediately before the instruction) ----


def split_waits(nc):
    nsplit = 0
    blocks = list(nc.main_func.blocks)
    for blk in blocks:
        new_list = []
        for ins in list(blk.instructions):
            si = getattr(ins, "sync_info", None)
            if si is not None and si.on_wait and len(si.on_wait) > 1:
                waits = list(si.on_wait)
                eng = ins.engine
                for w in waits[:-1]:
                    nop = nc.engines[eng].nop(hint="waitsplit", nofuse=True).ins
                    _remove(nc, nop)
                    nop.sync_info = mybir.SyncInfo(on_wait=[w], on_update=[])
                    new_list.append(nop)
                    nsplit += 1
                ins.sync_info = mybir.SyncInfo(
                    on_wait=[waits[-1]], on_update=list(si.on_update or [])
                )
            new_list.append(ins)
        blk.instructions[:] = new_list
    return nsplit


def _remove(nc, ins):
    for blk in nc.main_func.blocks:
        try:
            blk.instructions.remove(ins)
            return
        except ValueError:
            continue
    raise RuntimeError("freshly created nop not found in any block")


NCORES = 8
B, N, M = 64, 16384, 128
S = B // NCORES          # samples per core
K = 5                    # feature rows per chunk
CPG = 8                  # chunks per pair-block (stationary width 128 pts x 8)
PAIRS = 16               # pair-blocks per sample (16*8 = 128 chunks)
CH = N // 128            # 128-point chunks per sample (128)
COLS = S * CH            # minsq/logit columns per core (1024)
WPS = PAIRS // 2         # drain windows per sample (2 pairs each)
PEN = 100.0
HPAT = (('s', 'm', 'm', '4'), ('s', 'm', '4', '4'))  # window flavors by half
SOFTK = 700.0            # softmin sharpness for 's' windows

F = mybir.ActivationFunctionType
OP = mybir.AluOpType
DT = mybir.dt

_CACHE = {}


def build_nc(Mk):
    Mh = Mk // 2
    GW = CPG * Mh            # cols per A- or B-group (384 for Mk=96)
    nc = bass.Bass()
    lhsT = nc.declare_dram_parameter(
        "lhsT", [S, K * CPG, PAIRS * 128], DT.float16, isOutput=False
    )
    rhs = nc.declare_dram_parameter(
        "rhs", [K * CPG, S * 2 * GW], DT.float16, isOutput=False
    )
    lg = nc.declare_dram_parameter("lg", [128, COLS], DT.float32, isOutput=False)
    out = nc.declare_dram_parameter("out", [128, 8], DT.float32, isOutput=True)

    # bf16 min-tree levels: Mh -> ... -> wlast (tensor_reduce finishes)
    levels = []
    m = Mh
    while m % 2 == 0 and m > 3:
        m //= 2
        levels.append(m)

    with tile.TileContext(nc) as tc:
        with (
            tc.tile_pool(name="persist", bufs=1) as pp,
            tc.tile_pool(name="lhs", bufs=2) as lp,
            tc.tile_pool(name="abuf", bufs=4) as ap_,
            tc.tile_pool(name="pmpool", bufs=2) as pmp,
            tc.tile_pool(name="treep", bufs=2) as trp,
            tc.tile_pool(name="psum", bufs=1, space="PSUM") as psp,
        ):
            # --- input staging: sample-0 pieces first so PE ramps fast
            rt = pp.tile([K * CPG, S * 2 * GW], DT.float16)
            nc.sync.dma_start(out=rt[:, 0:2 * GW], in_=rhs[:, 0:2 * GW])
            lts = [lp.tile([K * CPG, PAIRS * 128], DT.float16, tag="lhsT",
                           name=f"lt{i}") for i in range(2)]
            nc.sync.dma_start(out=lts[0][:, 0:256], in_=lhsT[0, :, 0:256])
            nc.sync.dma_start(out=lts[0][:, 256:1024], in_=lhsT[0, :, 256:1024])
            nc.sync.dma_start(out=lts[0][:, 1024:], in_=lhsT[0, :, 1024:])
            nc.sync.dma_start(out=rt[:, 2 * GW:], in_=rhs[:, 2 * GW:])
            lgt = pp.tile([128, COLS], DT.float32)

            ps = psp.tile([128, 4096], DT.float32)   # 8-slot ring, 512 each
            sums = pp.tile([128, 8], DT.float32)
            minsq = pp.tile([128, COLS], DT.float32)
            sp = pp.tile([128, COLS], DT.float32)
            ce = pp.tile([128, COLS], DT.float32)
            fo = pp.tile([128, COLS], DT.float32)
            wq = pp.tile([128, COLS // 4], DT.float32)

            def emit_sample(s):
                if s + 1 < S:
                    nc.sync.dma_start(out=lts[(s + 1) % 2][:], in_=lhsT[s + 1])
                lt = lts[s % 2]
                pm = pmp.tile([128, 48 * Mh], DT.bfloat16, tag="pm")
                pmoff = [0]
                rA = rt[:, s * 2 * GW: s * 2 * GW + GW]
                rB = rt[:, s * 2 * GW + GW: s * 2 * GW + 2 * GW]
                for w in range(WPS):
                    base = (w % 2) * 2048
                    for i in range(2):           # two pairs per window
                        p = w * 2 + i
                        sl = lt[:, p * 128:(p + 1) * 128]
                        nc.tensor.matmul(
                            out=ps[:, base + i * 1024: base + i * 1024 + GW],
                            lhsT=sl, rhs=rA, start=True, stop=True)
                        nc.tensor.matmul(
                            out=ps[:, base + i * 1024 + 512:
                                   base + i * 1024 + 512 + GW],
                            lhsT=sl, rhs=rB, start=True, stop=True)
                    h, pos = w // 4, w % 4
                    flavor = HPAT[h % 2][pos]
                    j = base // 1024
                    psv = ps[:].rearrange("p (g t x) -> p g t x", t=2, x=512)
                    if flavor == '4':
                        # P4: single DVE reduce over (half, m) -> minsq direct
                        pin = psv[:, j:j + 2, :, 0:GW].rearrange(
                            "p g t (v m) -> p g v t m", m=Mh)
                        c0 = s * CH + h * 64 + pos * 16
                        nc.vector.tensor_reduce(
                            out=minsq[:, c0:c0 + 16].rearrange(
                                "p (g v) -> p g v", g=2),
                            in_=pin, axis=mybir.AxisListType.XY, op=OP.min)
                    elif flavor == 's':
                        # soft: ACT exp(-K d2) -> bf16; POOL add-tree; DVE
                        # reduce-add -> S in minsq (log-finished per sample)
                        ph = ap_.tile([128, 4 * GW], DT.bfloat16, tag="ph4")
                        nc.scalar.activation(
                            ph[:].rearrange("p (g x) -> p g x", g=4),
                            ps[:, base:base + 2048].rearrange(
                                "p (g x) -> p g x", g=4)[:, :, 0:GW],
                            F.Exp, scale=-SOFTK)
                        pv4 = ph[:].rearrange("p (i t x) -> p i t x", i=2, t=2)
                        st = trp.tile([128, 16 * Mh], DT.bfloat16, tag="st0")
                        nc.gpsimd.tensor_tensor(
                            out=st[:].rearrange("p (i x) -> p i x", i=2),
                            in0=pv4[:, :, 0], in1=pv4[:, :, 1], op=OP.add)
                        scur = st[:].rearrange("p (c m) -> p c m", m=Mh)
                        for wn in levels:
                            sn = trp.tile([128, 16 * wn], DT.bfloat16,
                                          tag=f"st{wn}", name=f"s{wn}")
                            nc.gpsimd.tensor_tensor(
                                out=sn[:].rearrange("p (c m) -> p c m", m=wn),
                                in0=scur[:, :, 0:wn],
                                in1=scur[:, :, wn:2 * wn], op=OP.add)
                            scur = sn[:].rearrange("p (c m) -> p c m", m=wn)
                        c0 = s * CH + h * 64 + pos * 16
                        nc.vector.tensor_reduce(
                            out=minsq[:, c0:c0 + 16], in_=scur,
                            axis=mybir.AxisListType.X, op=OP.add)
                    else:
                        pmw = pm[:, pmoff[0] * Mh:(pmoff[0] + 16) * Mh
                                 ].rearrange("p (i g) -> p i g", g=GW)
                        pmoff[0] += 16
                        # P2: ACT copies all 4 slots; DVE bf16 2x min
                        ab = ap_.tile([128, 4 * GW], DT.bfloat16, tag="ab4")
                        nc.scalar.activation(
                            ab[:].rearrange("p (g x) -> p g x", g=4),
                            ps[:, base:base + 2048].rearrange(
                                "p (g x) -> p g x", g=4)[:, :, 0:GW],
                            F.Copy)
                        av = ab[:].rearrange("p (i h x) -> p i h x",
                                             i=2, h=2)
                        nc.vector.tensor_tensor(
                            out=pmw, in0=av[:, :, 0], in1=av[:, :, 1],
                            op=OP.min)
                    if w == WPS - 1:             # per-sample finishers
                        # min-tree over this sample's compact pm (48 chunks)
                        cur = pm[:, 0:48 * Mh].rearrange(
                            "p (c m) -> p c m", m=Mh)
                        for wnext in levels:
                            nxt = trp.tile([128, 48 * wnext], DT.bfloat16,
                                           tag=f"tr{wnext}", name=f"t{wnext}")
                            nc.vector.tensor_tensor(
                                out=nxt[:].rearrange("p (c m) -> p c m",
                                                     m=wnext),
                                in0=cur[:, :, 0:wnext],
                                in1=cur[:, :, wnext:2 * wnext],
                                op=OP.min)
                            cur = nxt[:].rearrange("p (c m) -> p c m", m=wnext)
                        c0 = s * CH
                        nc.vector.tensor_reduce(
                            out=minsq[:, c0 + 16:c0 + 48],
                            in_=cur[:, 0:32], axis=mybir.AxisListType.X,
                            op=OP.min)
                        nc.vector.tensor_reduce(
                            out=minsq[:, c0 + 80:c0 + 96],
                            in_=cur[:, 32:48], axis=mybir.AxisListType.X,
                            op=OP.min)
                        # soft windows: minsq_soft = -ln(S)/K (strided cols)
                        sv = minsq[:, c0:c0 + 128].rearrange(
                            "p (h x) -> p h x", h=2)[:, :, 0:16]
                        nc.scalar.activation(sv, sv, F.Ln)
                        nc.scalar.activation(sv, sv, F.Copy,
                                             scale=-1.0 / SOFTK)

            def emit_focal():
                nc.scalar.activation(sp[:], lgt[:], F.Exp, scale=-1.0)
                nc.scalar.activation(sp[:], sp[:], F.Ln, bias=1.0)  # sp(-y)
                nc.gpsimd.tensor_tensor(out=ce[:], in0=lgt[:], in1=sp[:],
                                        op=OP.add)                  # sp(y)
                nc.scalar.activation(sp[:], sp[:], F.Exp, scale=-2.0)  # u^2
                nc.gpsimd.tensor_tensor(out=fo[:], in0=ce[:], in1=sp[:],
                                        op=OP.mult)
                nc.vector.tensor_reduce(
                    out=sums[:, 0:1], in_=fo[:], axis=mybir.AxisListType.X,
                    op=OP.add)

            def emit_epilogue(q):
                c0, c1 = q * (COLS // 4), (q + 1) * (COLS // 4)
                ms = minsq[:, c0:c1]
                nc.vector.tensor_scalar_max(out=ms, in0=ms, scalar1=1e-12)
                nc.scalar.activation(ms, ms, F.Ln)
                nc.scalar.activation(ms, ms, F.Exp, scale=0.5)    # sqrt
                nc.scalar.activation(ms, ms, F.Exp, scale=-10.0)  # w
                nc.gpsimd.tensor_tensor(out=wq[:], in0=fo[:, c0:c1], in1=ms,
                                        op=OP.mult)
                nc.vector.tensor_reduce(
                    out=sums[:, 4 + q:5 + q], in_=wq[:],
                    axis=mybir.AxisListType.X, op=OP.add)

            emit_sample(0)
            nc.sync.dma_start(out=lgt[:], in_=lg[:])
            emit_focal()
            for s in range(1, S):
                emit_sample(s)
                if s in (1, 3, 5):
                    emit_epilogue(s // 2)
            emit_epilogue(3)
            nc.sync.dma_start(out=out[:], in_=sums[:])

    split_waits(nc)
    return nc


def pack_inputs(inputs, targets, point_coords, corner_coords):
    """Host-side shard + layout packing. Returns (in_maps, Mk)."""
    x = np.asarray(inputs, np.float32)
    t = np.asarray(targets, np.float32)
    pc = np.asarray(point_coords, np.float32)
    cc = np.asarray(corner_coords, np.float32)

    pts = pc[..., :3]
    q = (pts * pts).sum(-1)
    feats = np.empty((B, K, N), np.float32)
    feats[:, 0] = pts[..., 0]
    feats[:, 1] = pts[..., 1]
    feats[:, 2] = pts[..., 2]
    feats[:, 3] = q
    feats[:, 4] = 1.0
    # [B, K, PAIRS, CPG, 128] -> [B, CPG, K, PAIRS, 128] -> [B, 40, 2048]
    fg = feats.reshape(B, K, PAIRS, CPG, 128).transpose(0, 3, 1, 2, 4)
    lhsT = fg.reshape(B, K * CPG, PAIRS * 128).astype(np.float16)

    # corners: compact valid to front, pad with PEN sentinels at origin
    valid = cc[..., 0] > -1.0
    nv = valid.sum(-1)
    maxv = int(nv.max()) if nv.max() > 0 else 1
    Mk = min(M, ((maxv + 31) // 32) * 32)
    Mh = Mk // 2
    GW = CPG * Mh
    cfeat = np.zeros((B, K, Mk), np.float32)
    cfeat[:, 4] = PEN
    for b in range(B):
        v = cc[b][valid[b]]
        n = v.shape[0]
        cfeat[b, 0, :n] = -2.0 * v[:, 0]
        cfeat[b, 1, :n] = -2.0 * v[:, 1]
        cfeat[b, 2, :n] = -2.0 * v[:, 2]
        cfeat[b, 3, :n] = 1.0
        cfeat[b, 4, :n] = (v * v).sum(-1)
    fA = cfeat[:, :, 0:Mh]                       # [B, K, Mh]
    fB = cfeat[:, :, Mh:Mk]                      # [B, K, Mh]
    rhsf = np.zeros((B, K * CPG, 2 * GW), np.float32)
    for v in range(CPG):
        rhsf[:, v * K:(v + 1) * K, v * Mh:(v + 1) * Mh] = fA
        rhsf[:, v * K:(v + 1) * K, GW + v * Mh:GW + (v + 1) * Mh] = fB
    rhsf = rhsf.astype(np.float16)

    yh = ((1.0 - 2.0 * t) * x).astype(np.float32)  # focal depends only on y

    in_maps = []
    for c in range(NCORES):
        sl = slice(c * S, (c + 1) * S)
        lgp = yh[sl].reshape(S, CH, 128).transpose(2, 0, 1).reshape(
            128, COLS).copy()
        rhp = rhsf[sl].transpose(1, 0, 2).reshape(K * CPG, S * 2 * GW).copy()
        in_maps.append({
            "lhsT": np.ascontiguousarray(lhsT[sl]),
            "rhs": rhp,
            "lg": lgp,
        })
    return in_maps, Mk


def _finalize(results):
    s1 = 0.0
    s2 = 0.0
    for r in results:
        o = np.asarray(r["out"], np.float64)
        s1 += o[:, 0].sum()
        s2 += o[:, 4:8].sum()
    bn = float(B * N)
    focal = s1 / bn
    distance = (s1 + 2.0 * s2) / bn
    total = focal + distance
    return (np.float32(total), np.float32(focal), np.float32(distance))


def kernel(inputs, targets, point_coords, corner_coords):
    in_maps, Mk = pack_inputs(inputs, targets, point_coords, corner_coords)
    if Mk not in _CACHE:
        _CACHE[Mk] = build_nc(Mk)
    nc = _CACHE[Mk]
    res = run_bass_kernel_spmd(nc, in_maps, core_ids=list(range(NCORES)))
    return _finalize(res.results)


if __name__ == "__main__":
    rng = np.random.default_rng(0)
    ins = {
        "inputs": rng.standard_normal((B, N), dtype=np.float32),
        "targets": (rng.random((B, N)) < 0.05).astype(np.float32),
        "point_coords": rng.random((B, N, 6), dtype=np.float32),
        "corner_coords": rng.random((B, 128, 3), dtype=np.float32),
    }
    print(kernel(**ins))


# revision 10
# speedup vs baseline: 1.0059x; 1.0059x over previous
"""AdaptiveCornerLoss on 8 TRN2 NeuronCores — batch-parallel Bass/Tile kernel.

Shapes (hardcoded): B=64, N=16384, C=6, M=128 corners. 8 cores, 8 samples/core.

Math:
  focal    = u^2 * ce  with  y=(1-2t)*x, ce=softplus(y)=y+ln(1+e^-y),
             u=sigmoid(y)  =>  u^2 = exp(-2*ln(1+e^{-y}))
  d2(n,m)  = |p|^2 + |c|^2 - 2 p.c   (fp16 matmul; point rows [px,py,pz,|p|^2,1]
             vs corner rows [-2cx,-2cy,-2cz,1,|c|^2+pen])
  w        = exp(-10*sqrt(max(min_m d2, 1e-12))) via exp/ln on ACT

v2 design notes (measured on hw via probe):
  * PE output port is the hard floor: 128 fp32 PSUM values per 0.833ns
    cycle regardless of matmul shape (~82us/core for N*Mk d2 values).
    One 128-wide stationary [40,128] per pair of A/B groups; LDW (143ns)
    hides under the 320ns streams.
  * Corners split into A-half [0:Mh) / B-half [Mh:Mk); PE emits them as
    SEPARATE 384-col PSUM groups (8-slot ring of 512-col slots). This
    decouples ACT (copies A and/or B to SBUF bf16) from DVE (pairmin
    TT-min), since a TT may read only ONE operand from PSUM and GPSIMD
    cannot read PSUM at all (nor do min).
  * Drain windows = 2 pairs (4 groups, 16 chunks). Two flavors:
      P2 window: ACT copies all 4 groups -> bf16; DVE TT-min in 2x mode.
      P1 window: ACT copies A-groups only; DVE TT-min(B-PSUM, A-sbuf).
    The mix balances ACT vs DVE load (tunable P2MOD/P2CNT).
  * bf16 min-tree per half-sample batch on DVE (2x mode), final 3-wide
    tensor_reduce -> minsq fp32.
  * GPSIMD (Pool) handles focal ce/fo and the epilogue multiply; ACT the
    exp/ln chains.
Outputs per core: per-partition partial sums [128,8]: col0 = sum(focal),
cols 4:6 = half-sums of focal*w; host reduces and forms the 3 losses.
"""

import sys

sys.path.insert(0, "/opt/trn_rl_repo")

import numpy as np

import concourse.bass as bass
import concourse.mybir as mybir
from concourse import tile
from concourse.bass_utils import run_bass_kernel_spmd

# ---- inlined wait-splitter (this neuronx-cc build allows at most ONE sync
# wait per instruction; Tile emits multi-waits freely, so excess waits move
# onto nofuse NOPs on the same engine immediately before the instruction) ----


def split_waits(nc):
    nsplit = 0
    blocks = list(nc.main_func.blocks)
    for blk in blocks:
        new_list = []
        for ins in list(blk.instructions):
            si = getattr(ins, "sync_info", None)
            if si is not None and si.on_wait and len(si.on_wait) > 1:
                waits = list(si.on_wait)
                eng = ins.engine
                for w in waits[:-1]:
                    nop = nc.engines[eng].nop(hint="waitsplit", nofuse=True).ins
                    _remove(nc, nop)
                    nop.sync_info = mybir.SyncInfo(on_wait=[w], on_update=[])
                    new_list.append(nop)
                    nsplit += 1
                ins.sync_info = mybir.SyncInfo(
                    on_wait=[waits[-1]], on_update=list(si.on_update or [])
                )
            new_list.append(ins)
        blk.instructions[:] = new_list
    return nsplit


def _remove(nc, ins):
    for blk in nc.main_func.blocks:
        try:
            blk.instructions.remove(ins)
            return
        except ValueError:
            continue
    raise RuntimeError("freshly created nop not found in any block")


NCORES = 8
B, N, M = 64, 16384, 128
S = B // NCORES          # samples per core
K = 5                    # feature rows per chunk
CPG = 8                  # chunks per pair-block (stationary width 128 pts x 8)
PAIRS = 16               # pair-blocks per sample (16*8 = 128 chunks)
CH = N // 128            # 128-point chunks per sample (128)
COLS = S * CH            # minsq/logit columns per core (1024)
WPS = PAIRS // 2         # drain windows per sample (2 pairs each)
PEN = 100.0
HPAT = (('s', 'm', 'm', '4'), ('s', 'm', '4', '4'))  # window flavors by half
SOFTK = 700.0            # softmin sharpness for 's' windows

F = mybir.ActivationFunctionType
OP = mybir.AluOpType
DT = mybir.dt

_CACHE = {}


def build_nc(Mk):
    Mh = Mk // 2
    GW = CPG * Mh            # cols per A- or B-group (384 for Mk=96)
    nc = bass.Bass()
    lhsT = nc.declare_dram_parameter(
        "lhsT", [S, K * CPG, PAIRS * 128], DT.float16, isOutput=False
    )
    rhs = nc.declare_dram_parameter(
        "rhs", [K * CPG, S * 2 * GW], DT.float16, isOutput=False
    )
    lg = nc.declare_dram_parameter("lg", [128, COLS], DT.float32, isOutput=False)
    out = nc.declare_dram_parameter("out", [128, 8], DT.float32, isOutput=True)

    # bf16 min-tree levels: Mh -> ... -> wlast (tensor_reduce finishes)
    levels = []
    m = Mh
    while m % 2 == 0 and m > 3:
        m //= 2
        levels.append(m)

    with tile.TileContext(nc) as tc:
        with (
            tc.tile_pool(name="persist", bufs=1) as pp,
            tc.tile_pool(name="lhs", bufs=2) as lp,
            tc.tile_pool(name="abuf", bufs=4) as ap_,
            tc.tile_pool(name="pmpool", bufs=2) as pmp,
            tc.tile_pool(name="treep", bufs=2) as trp,
            tc.tile_pool(name="psum", bufs=1, space="PSUM") as psp,
        ):
            # --- input staging: sample-0 pieces first so PE ramps fast
            rt = pp.tile([K * CPG, S * 2 * GW], DT.float16)
            nc.sync.dma_start(out=rt[:, 0:2 * GW], in_=rhs[:, 0:2 * GW])
            lts = [lp.tile([K * CPG, PAIRS * 128], DT.float16, tag="lhsT",
                           name=f"lt{i}") for i in range(2)]
            nc.sync.dma_start(out=lts[0][:, 0:256], in_=lhsT[0, :, 0:256])
            nc.sync.dma_start(out=lts[0][:, 256:1024], in_=lhsT[0, :, 256:1024])
            nc.sync.dma_start(out=lts[0][:, 1024:], in_=lhsT[0, :, 1024:])
            nc.sync.dma_start(out=rt[:, 2 * GW:], in_=rhs[:, 2 * GW:])
            lgt = pp.tile([128, COLS], DT.float32)

            ps = psp.tile([128, 4096], DT.float32)   # 8-slot ring, 512 each
            sums = pp.tile([128, 8], DT.float32)
            minsq = pp.tile([128, COLS], DT.float32)
            sp = pp.tile([128, COLS], DT.float32)
            ce = pp.tile([128, COLS], DT.float32)
            fo = pp.tile([128, COLS], DT.float32)
            wq = pp.tile([128, COLS // 4], DT.float32)

            def emit_sample(s):
                if s + 1 < S:
                    nc.sync.dma_start(out=lts[(s + 1) % 2][:], in_=lhsT[s + 1])
                lt = lts[s % 2]
                pm = pmp.tile([128, 48 * Mh], DT.bfloat16, tag="pm")
                pmoff = [0]
                rA = rt[:, s * 2 * GW: s * 2 * GW + GW]
                rB = rt[:, s * 2 * GW + GW: s * 2 * GW + 2 * GW]
                for w in range(WPS):
                    base = (w % 2) * 2048
                    for i in range(2):           # two pairs per window
                        p = w * 2 + i
                        sl = lt[:, p * 128:(p + 1) * 128]
                        nc.tensor.matmul(
                            out=ps[:, base + i * 1024: base + i * 1024 + GW],
                            lhsT=sl, rhs=rA, start=True, stop=True)
                        nc.tensor.matmul(
                            out=ps[:, base + i * 1024 + 512:
                                   base + i * 1024 + 512 + GW],
                            lhsT=sl, rhs=rB, start=True, stop=True)
                    h, pos = w // 4, w % 4
                    flavor = HPAT[h % 2][pos]
                    j = base // 1024
                    psv = ps[:].rearrange("p (g t x) -> p g t x", t=2, x=512)
                    if flavor == '4':
                        # P4: single DVE reduce over (half, m) -> minsq direct
                        pin = psv[:, j:j + 2, :, 0:GW].rearrange(
                            "p g t (v m) -> p g v t m", m=Mh)
                        c0 = s * CH + h * 64 + pos * 16
                        nc.vector.tensor_reduce(
                            out=minsq[:, c0:c0 + 16].rearrange(
                                "p (g v) -> p g v", g=2),
                            in_=pin, axis=mybir.AxisListType.XY, op=OP.min)
                    elif flavor == 's':
                        # soft: ACT exp(-K d2) -> bf16; POOL add-tree; DVE
                        # reduce-add -> S in minsq (log-finished per sample)
                        ph = ap_.tile([128, 4 * GW], DT.bfloat16, tag="ph4")
                        nc.scalar.activation(
                            ph[:].rearrange("p (g x) -> p g x", g=4),
                            ps[:, base:base + 2048].rearrange(
                                "p (g x) -> p g x", g=4)[:, :, 0:GW],
                            F.Exp, scale=-SOFTK)
                        pv4 = ph[:].rearrange("p (i t x) -> p i t x", i=2, t=2)
                        st = trp.tile([128, 16 * Mh], DT.bfloat16, tag="st0")
                        nc.gpsimd.tensor_tensor(
                            out=st[:].rearrange("p (i x) -> p i x", i=2),
                            in0=pv4[:, :, 0], in1=pv4[:, :, 1], op=OP.add)
                        scur = st[:].rearrange("p (c m) -> p c m", m=Mh)
                        for wn in levels[:2]:
                            sn = trp.tile([128, 16 * wn], DT.bfloat16,
                                          tag=f"st{wn}", name=f"s{wn}")
                            nc.gpsimd.tensor_tensor(
                                out=sn[:].rearrange("p (c m) -> p c m", m=wn),
                                in0=scur[:, :, 0:wn],
                                in1=scur[:, :, wn:2 * wn], op=OP.add)
                            scur = sn[:].rearrange("p (c m) -> p c m", m=wn)
                        c0 = s * CH + h * 64 + pos * 16
                        nc.vector.tensor_reduce(
                            out=minsq[:, c0:c0 + 16], in_=scur,
                            axis=mybir.AxisListType.X, op=OP.add)
                        del scur
                    else:
                        pmw = pm[:, pmoff[0] * Mh:(pmoff[0] + 16) * Mh
                                 ].rearrange("p (i g) -> p i g", g=GW)
                        pmoff[0] += 16
                        # P2: ACT copies all 4 slots; DVE bf16 2x min
                        ab = ap_.tile([128, 4 * GW], DT.bfloat16, tag="ab4")
                        nc.scalar.activation(
                            ab[:].rearrange("p (g x) -> p g x", g=4),
                            ps[:, base:base + 2048].rearrange(
                                "p (g x) -> p g x", g=4)[:, :, 0:GW],
                            F.Copy)
                        av = ab[:].rearrange("p (i h x) -> p i h x",
                                             i=2, h=2)
                        nc.vector.tensor_tensor(
                            out=pmw, in0=av[:, :, 0], in1=av[:, :, 1],
                            op=OP.min)
                    if w == WPS - 1:             # per-sample finishers
                        # min-tree over this sample's compact pm (48 chunks)
                        cur = pm[:, 0:48 * Mh].rearrange(
                            "p (c m) -> p c m", m=Mh)
                        for wnext in levels:
                            nxt = trp.tile([128, 48 * wnext], DT.bfloat16,
                                           tag=f"tr{wnext}", name=f"t{wnext}")
                            nc.vector.tensor_tensor(
                                out=nxt[:].rearrange("p (c m) -> p c m",
                                                     m=wnext),
                                in0=cur[:, :, 0:wnext],
                                in1=cur[:, :, wnext:2 * wnext],
                                op=OP.min)
                            cur = nxt[:].rearrange("p (c m) -> p c m", m=wnext)
                        c0 = s * CH
                        nc.vector.tensor_reduce(
                            out=minsq[:, c0 + 16:c0 + 48],
                            in_=cur[:, 0:32], axis=mybir.AxisListType.X,
                            op=OP.min)
                        nc.vector.tensor_reduce(
                            out=minsq[:, c0 + 80:c0 + 96],
                            in_=cur[:, 32:48], axis=mybir.AxisListType.X,
                            op=OP.min)
                        # soft windows: minsq_soft = -ln(S)/K (strided cols)
                        sv = minsq[:, c0:c0 + 128].rearrange(
                            "p (h x) -> p h x", h=2)[:, :, 0:16]
                        nc.scalar.activation(sv, sv, F.Ln)
                        nc.scalar.activation(sv, sv, F.Copy,
                                             scale=-1.0 / SOFTK)

            def emit_focal():
                nc.scalar.activation(sp[:], lgt[:], F.Exp, scale=-1.0)
                nc.scalar.activation(sp[:], sp[:], F.Ln, bias=1.0)  # sp(-y)
                nc.gpsimd.tensor_tensor(out=ce[:], in0=lgt[:], in1=sp[:],
                                        op=OP.add)                  # sp(y)
                nc.scalar.activation(sp[:], sp[:], F.Exp, scale=-2.0)  # u^2
                nc.gpsimd.tensor_tensor(out=fo[:], in0=ce[:], in1=sp[:],
                                        op=OP.mult)
                nc.vector.tensor_reduce(
                    out=sums[:, 0:1], in_=fo[:], axis=mybir.AxisListType.X,
                    op=OP.add)

            def emit_epilogue(q):
                c0, c1 = q * (COLS // 4), (q + 1) * (COLS // 4)
                ms = minsq[:, c0:c1]
                nc.vector.tensor_scalar_max(out=ms, in0=ms, scalar1=1e-12)
                nc.scalar.activation(ms, ms, F.Ln)
                nc.scalar.activation(ms, ms, F.Exp, scale=0.5)    # sqrt
                nc.scalar.activation(ms, ms, F.Exp, scale=-10.0)  # w
                nc.gpsimd.tensor_tensor(out=wq[:], in0=fo[:, c0:c1], in1=ms,
                                        op=OP.mult)
                nc.vector.tensor_reduce(
                    out=sums[:, 4 + q:5 + q], in_=wq[:],
                    axis=mybir.AxisListType.X, op=OP.add)

            emit_sample(0)
            nc.sync.dma_start(out=lgt[:], in_=lg[:])
            emit_focal()
            for s in range(1, S):
                emit_sample(s)
                if s in (1, 3, 5):
                    emit_epilogue(s // 2)
            emit_epilogue(3)
            nc.sync.dma_start(out=out[:], in_=sums[:])

    split_waits(nc)
    return nc


def pack_inputs(inputs, targets, point_coords, corner_coords):
    """Host-side shard + layout packing. Returns (in_maps, Mk)."""
    x = np.asarray(inputs, np.float32)
    t = np.asarray(targets, np.float32)
    pc = np.asarray(point_coords, np.float32)
    cc = np.asarray(corner_coords, np.float32)

    pts = pc[..., :3]
    q = (pts * pts).sum(-1)
    feats = np.empty((B, K, N), np.float32)
    feats[:, 0] = pts[..., 0]
    feats[:, 1] = pts[..., 1]
    feats[:, 2] = pts[..., 2]
    feats[:, 3] = q
    feats[:, 4] = 1.0
    # [B, K, PAIRS, CPG, 128] -> [B, CPG, K, PAIRS, 128] -> [B, 40, 2048]
    fg = feats.reshape(B, K, PAIRS, CPG, 128).transpose(0, 3, 1, 2, 4)
    lhsT = fg.reshape(B, K * CPG, PAIRS * 128).astype(np.float16)

    # corners: compact valid to front, pad with PEN sentinels at origin
    valid = cc[..., 0] > -1.0
    nv = valid.sum(-1)
    maxv = int(nv.max()) if nv.max() > 0 else 1
    Mk = min(M, ((maxv + 31) // 32) * 32)
    Mh = Mk // 2
    GW = CPG * Mh
    cfeat = np.zeros((B, K, Mk), np.float32)
    cfeat[:, 4] = PEN
    for b in range(B):
        v = cc[b][valid[b]]
        n = v.shape[0]
        cfeat[b, 0, :n] = -2.0 * v[:, 0]
        cfeat[b, 1, :n] = -2.0 * v[:, 1]
        cfeat[b, 2, :n] = -2.0 * v[:, 2]
        cfeat[b, 3, :n] = 1.0
        cfeat[b, 4, :n] = (v * v).sum(-1)
    fA = cfeat[:, :, 0:Mh]                       # [B, K, Mh]
    fB = cfeat[:, :, Mh:Mk]                      # [B, K, Mh]
    rhsf = np.zeros((B, K * CPG, 2 * GW), np.float32)
    for v in range(CPG):
        rhsf[:, v * K:(v + 1) * K, v * Mh:(v + 1) * Mh] = fA
        rhsf[:, v * K:(v + 1) * K, GW + v * Mh:GW + (v + 1) * Mh] = fB
    rhsf = rhsf.astype(np.float16)

    yh = ((1.0 - 2.0 * t) * x).astype(np.float32)  # focal depends only on y

    in_maps = []
    for c in range(NCORES):
        sl = slice(c * S, (c + 1) * S)
        lgp = yh[sl].reshape(S, CH, 128).transpose(2, 0, 1).reshape(
            128, COLS).copy()
        rhp = rhsf[sl].transpose(1, 0, 2).reshape(K * CPG, S * 2 * GW).copy()
        in_maps.append({
            "lhsT": np.ascontiguousarray(lhsT[sl]),
            "rhs": rhp,
            "lg": lgp,
        })
    return in_maps, Mk


def _finalize(results):
    s1 = 0.0
    s2 = 0.0
    for r in results:
        o = np.asarray(r["out"], np.float64)
        s1 += o[:, 0].sum()
        s2 += o[:, 4:8].sum()
    bn = float(B * N)
    focal = s1 / bn
    distance = (s1 + 2.0 * s2) / bn
    total = focal + distance
    return (np.float32(total), np.float32(focal), np.float32(distance))


def kernel(inputs, targets, point_coords, corner_coords):
    in_maps, Mk = pack_inputs(inputs, targets, point_coords, corner_coords)
    if Mk not in _CACHE:
        _CACHE[Mk] = build_nc(Mk)
    nc = _CACHE[Mk]
    res = run_bass_kernel_spmd(nc, in_maps, core_ids=list(range(NCORES)))
    return _finalize(res.results)


if __name__ == "__main__":
    rng = np.random.default_rng(0)
    ins = {
        "inputs": rng.standard_normal((B, N), dtype=np.float32),
        "targets": (rng.random((B, N)) < 0.05).astype(np.float32),
        "point_coords": rng.random((B, N, 6), dtype=np.float32),
        "corner_coords": rng.random((B, 128, 3), dtype=np.float32),
    }
    print(kernel(**ins))
